# revision 24
# baseline (speedup 1.0000x reference)
"""Trainium2 Bass kernel for local windowed per-channel attention (sparse_attention).

Reference computation (per batch b, channel c, position (h,w)):
    q = W_q x ; k = W_k x_pad ; v = W_v x_pad           (1x1 convs)
    s[i,j]  = q[h,w] * (k[h+i, w+j] + bias[c, i or j])  over a 7x7 window
    out     = sum_ij softmax_ij(s) * v[h+i, w+j]

Sharding: spatial, 8 ways — core = (batch, 12-row slab). Fully independent
per core (no collectives). Host pre-pads each slab with the 3-row/col halo.

Per-core dataflow (channels on partitions, 2 channel-tiles of 128):
  TensorE : q/k/v GEMMs in float32r (1 cyc/row); 49-tap reduction of
            den/num via identity-matmul accumulation into PSUM (bf16 rhs,
            f32 accumulate) — [e|m] interleaved per tap so den|num share
            one 3-bank region at 3 matmuls/tap.
  VectorE : score mult q*kb and weight mult e*v in fp16/bf16 at the DVE 2x
            perf mode, via shifted-window access patterns (no unfold
            materialization). The per-(c,tap) rel-pos bias is pre-folded
            into 7 biased fp16 k maps (channel-tile 0 biases by kh,
            tile 1 by kw) with 4x-mode tensor_scalar adds. Window taps
            are grouped so the bias slab is fixed per instruction, and
            split into even/odd kw sets reading 1-column-shifted copies
            (kb1/v1) so every innermost fp16 run stays 4B-aligned.
  ScalarE : exp, in place on the e slots (fp16 scores -> bf16), plus PSUM
            evictions.
  GpSimd  : a tuned slice of the weight-mult work (the score mults stay on
            VectorE — exp waits on them and Pool is ~3.4x slower).
  DMA     : shifted-copy production (kb1/v1) besides I/O.
  out = num / den  (reciprocal + mult), DMA out.
"""
import os
import numpy as np

from concourse import bass, bacc, mybir, tile
from concourse.bass_utils import run_bass_kernel_spmd

F32 = mybir.dt.float32
F16 = mybir.dt.float16
BF16 = mybir.dt.bfloat16

K, PAD = 7, 3
B, CIN, COUT, H, W = 2, 256, 256, 48, 48
ROWS = 12                 # output rows per core
SH, SW = ROWS + 2 * PAD, W + 2 * PAD   # 18, 54 padded slab
NPOS = ROWS * W           # 576 output positions per core
NPAD = SH * SW            # 972 padded positions
NQ = ROWS * SW            # 648 q-map positions (12 rows x 54 cols)
N_CORES = 8

SCORE_DT = F16            # kb, q, s dtype (f16 -> DVE 2x mode)
E_DT = BF16               # exp output / matmul rhs dtype (needs bf16 range)
V_DT = F16
F32R = mybir.dt.float32r  # TensorE fp32-replicated: 1 cyc/row when n>=256
GEMM_F32R = bool(int(os.environ.get("GEMM_F32R", "1")))
# m-mult groups offloaded to GpSimd (score mults stay on DVE: exp waits on
# them and Pool is ~3.4x slower per group). Keys: mt0 -> (i, par); mt1 -> j.
POOL_M0 = {(i, 1) for i in range(K)} | {(2, 0), (5, 0)}
# mt1 m-mults are emitted in two i-halves (finer Pool quanta): (g, half)
POOL_M1 = {(1, 0), (1, 1), (4, 0), (4, 1)}
TAPW = 2 * NPOS           # per-tap [e | m] interleaved width
RING_BUFS = int(os.environ.get("RING_BUFS", "6"))
MT_ILV = bool(int(os.environ.get("MT_ILV", "0")))

JEVEN = [0, 2, 4, 6]
JODD = [1, 3, 5]

_CACHED = {}


def _fap(t, offset, dims):
    """Custom free-dim AP on a tile: dims = [[stride, size], ...]."""
    a = t[:]
    return bass.AP(a.tensor, a.offset + offset, [list(a.ap[0])] + dims)


def _emit_body(nc, tc, dram):
    x_d, wq_d, wk_d, wv_d, beta_d, id_d, out_d = dram
    MULT = mybir.AluOpType.mult
    with (
        tc.tile_pool(name="const", bufs=1) as const,
        tc.tile_pool(name="work", bufs=1) as work,
    ):
        # ---- load inputs (small/critical first) ----
        GDT = F32R if GEMM_F32R else F32
        beta_sb = const.tile([128, 2 * K], F32, name="beta_sb")
        nc.sync.dma_start(beta_sb[:], beta_d[:, :])
        idf = const.tile([128, 128], F32, name="idf")
        nc.sync.dma_start(idf[:], id_d[:, :])
        idb = const.tile([128, 128], E_DT, name="idb")
        nc.vector.tensor_copy(idb[:], idf[:])
        # warm the ACT exp table while DMAs stream in
        warm = const.tile([128, 2], F32, name="warm")
        nc.scalar.activation(warm[:], idf[:, :2],
                             mybir.ActivationFunctionType.Exp)
        x_sb = []
        for kt in range(2):
            t = const.tile([128, NPAD], GDT, name=f"x_sb{kt}")
            nc.sync.dma_start(t[:, :512], x_d[kt * 128:(kt + 1) * 128, :512])
            nc.sync.dma_start(t[:, 512:], x_d[kt * 128:(kt + 1) * 128, 512:])
            x_sb.append(t)
        w_sb = {}
        for nm, d in (("k", wk_d), ("q", wq_d), ("v", wv_d)):
            for kt in range(2):
                t = const.tile([128, COUT], GDT, name=f"w{nm}{kt}")
                nc.sync.dma_start(t[:], d[kt * 128:(kt + 1) * 128, :])
                w_sb[nm, kt] = t

        # ---- per channel-tile persistent tensors ----
        # kb0 = 7 pre-biased fp16 k maps (slab t: k + beta[c, t]); kb1 =
        # 1-col-shifted copy of the stack (odd-kw tap 4B alignment).
        k0 = [work.tile([128, NPAD], SCORE_DT, name=f"k0_{mt}")
              for mt in range(2)]
        kb0 = [work.tile([128, K * NPAD], SCORE_DT, name=f"kb0_{mt}")
               for mt in range(2)]
        kb1 = [work.tile([128, K * NPAD], SCORE_DT, name=f"kb1_{mt}")
               for mt in range(2)]
        v0 = [work.tile([128, NPAD], V_DT, name=f"v0_{mt}") for mt in range(2)]
        v1 = [work.tile([128, NPAD], V_DT, name=f"v1_{mt}") for mt in range(2)]
        q_sb = [work.tile([128, NPOS], SCORE_DT, name=f"q{mt}")
                for mt in range(2)]

        def gmm(out, lhsT, rhs, start, stop):
            nc.tensor.matmul(out, lhsT, rhs, start=start, stop=stop)

        # ---- GEMMs ----
        with tc.tile_pool(name="gpsum", bufs=3, space="PSUM") as gpsum:
            for mt in range(2):
                mm = slice(mt * 128, (mt + 1) * 128)
                # k map
                kp = gpsum.tile([128, NPAD], F32, tag="gp", name=f"kp{mt}")
                for kt in range(2):
                    for c0, c1 in ((0, 512), (512, NPAD)):
                        gmm(kp[:, c0:c1], w_sb["k", kt][:, mm],
                            x_sb[kt][:, c0:c1],
                            start=(kt == 0), stop=(kt == 1))
                nc.scalar.copy(k0[mt][:], kp[:])
                # biased stack: fp16 tensor_scalar adds run the 4x DVE mode;
                # per-slab shifted copies (DMA) unblock early tap groups
                for t in range(K):
                    nc.vector.tensor_scalar_add(
                        kb0[mt][:, t * NPAD:(t + 1) * NPAD], k0[mt][:],
                        beta_sb[:, mt * K + t:mt * K + t + 1])
                    nc.sync.dma_start(
                        kb1[mt][:, t * NPAD:t * NPAD + NPAD - 2],
                        kb0[mt][:, t * NPAD + 1:t * NPAD + NPAD - 1])
                # q map: only the 12 center rows (cols incl. pad)
                qp = gpsum.tile([128, NQ], F32, tag="gp", name=f"qp{mt}")
                for kt in range(2):
                    for c0, c1 in ((0, 512), (512, NQ)):
                        gmm(qp[:, c0:c1], w_sb["q", kt][:, mm],
                            x_sb[kt][:, PAD * SW + c0:PAD * SW + c1],
                            start=(kt == 0), stop=(kt == 1))
                # evict q to dense [128, 12*48] fp16 (drop col pad)
                nc.vector.tensor_copy(
                    q_sb[mt][:].rearrange("p (h w) -> p h w", h=ROWS),
                    _fap(qp, PAD, [[SW, ROWS], [1, W]]))
                # v map
                vp = gpsum.tile([128, NPAD], F32, tag="gp", name=f"vp{mt}")
                for kt in range(2):
                    for c0, c1 in ((0, 512), (512, NPAD)):
                        gmm(vp[:, c0:c1], w_sb["v", kt][:, mm],
                            x_sb[kt][:, c0:c1],
                            start=(kt == 0), stop=(kt == 1))
                nc.scalar.copy(v0[mt][:], vp[:])
                nc.sync.dma_start(v1[mt][:, :NPAD - 2],
                                  v0[mt][:, 1:NPAD - 1])

        # ---- attention ----
        # Per-tap layout em_t[:, j*TAPW + (0:NPOS | NPOS:TAPW)] = [e_j | m_j]
        # so den|num accumulate into ONE [128, TAPW] PSUM region (3 banks)
        # with 3 matmuls per tap (512 | 512 | 128 cols).
        with (
            tc.tile_pool(name="apsum", bufs=1, space="PSUM") as apsum,
            tc.tile_pool(name="ring", bufs=RING_BUFS) as ring,
        ):
            dn_ps = [apsum.tile([128, TAPW], F32, name=f"dn{mt}")
                     for mt in range(2)]
            # channel-tile phases run sequentially: interleaving them was
            # measured slower (ring-slot and PSUM contention outweigh the
            # overlap; tile1's GEMMs already overlap tile0's attention).
            order = [(g, mt) for mt in range(2) for g in range(K)]
            done = set()
            for g, mt in order:
                    # group g: kh=i for tile0, kw=j for tile1 (bias slab g).
                    # Scores are written straight into the e-slots of em_t
                    # (as fp16 via bitcast) and exp'd in place (bf16 out).
                    em_t = ring.tile([128, K * TAPW], E_DT, tag="em",
                                     name=f"em{mt}_{g}")
                    if mt == 0:
                        # slots = j; even/odd kw split for fp16 alignment
                        for par, jl in ((0, JEVEN), (1, JODD)):
                            nj = len(jl)
                            kb_ap = _fap(kb1[0] if par else kb0[0],
                                         g * NPAD + g * SW + (jl[0] - par),
                                         [[2, nj], [SW, ROWS], [1, W]])
                            q_ap = _fap(q_sb[0], 0,
                                        [[0, nj], [W, ROWS], [1, W]])
                            s_ap = _fap(em_t, par * TAPW,
                                        [[2 * TAPW, nj], [W, ROWS], [1, W]]
                                        ).bitcast(SCORE_DT)
                            nc.vector.tensor_tensor(s_ap, kb_ap, q_ap, MULT)
                    else:
                        # slots = i; kw=g fixed -> one instr, parity by g
                        par = g % 2
                        kb_ap = _fap(kb1[1] if par else kb0[1],
                                     g * NPAD + (g - par),
                                     [[SW, K], [SW, ROWS], [1, W]])
                        q_ap = _fap(q_sb[1], 0, [[0, K], [W, ROWS], [1, W]])
                        s_ap = _fap(em_t, 0, [[TAPW, K], [W, ROWS], [1, W]]
                                    ).bitcast(SCORE_DT)
                        nc.vector.tensor_tensor(s_ap, kb_ap, q_ap, MULT)
                    # e = exp(s) in place on the e slots
                    nc.scalar.activation(
                        _fap(em_t, 0, [[TAPW, K], [1, NPOS]]),
                        _fap(em_t, 0, [[TAPW, K], [1, NPOS]]
                             ).bitcast(SCORE_DT),
                        mybir.ActivationFunctionType.Exp)
                    # m = e * v_shift -> m slots of em_t
                    if mt == 0:
                        for par, jl in ((0, JEVEN), (1, JODD)):
                            nj = len(jl)
                            e_ap = _fap(em_t, par * TAPW,
                                        [[2 * TAPW, nj], [W, ROWS], [1, W]])
                            v_ap = _fap(v1[0] if par else v0[0],
                                        g * SW + (jl[0] - par),
                                        [[2, nj], [SW, ROWS], [1, W]])
                            m_ap = _fap(em_t, par * TAPW + NPOS,
                                        [[2 * TAPW, nj], [W, ROWS], [1, W]])
                            eng = (nc.gpsimd if (g, par) in POOL_M0
                                   else nc.vector)
                            eng.tensor_tensor(m_ap, e_ap, v_ap, MULT)
                    else:
                        par = g % 2
                        for half, (i0, ni) in enumerate(((0, 4), (4, 3))):
                            e_ap = _fap(em_t, i0 * TAPW,
                                        [[TAPW, ni], [W, ROWS], [1, W]])
                            v_ap = _fap(v1[1] if par else v0[1],
                                        g - par + i0 * SW,
                                        [[SW, ni], [SW, ROWS], [1, W]])
                            m_ap = _fap(em_t, i0 * TAPW + NPOS,
                                        [[TAPW, ni], [W, ROWS], [1, W]])
                            eng = (nc.gpsimd if (g, half) in POOL_M1
                                   else nc.vector)
                            eng.tensor_tensor(m_ap, e_ap, v_ap, MULT)
                    # accumulate [den | num] += [e | m] via identity matmul
                    first = (g == 0)
                    last = (g == K - 1)
                    for sl in range(K):
                        for c0, c1 in ((0, 512), (512, 1024), (1024, TAPW)):
                            nc.tensor.matmul(
                                dn_ps[mt][:, c0:c1], idb[:],
                                em_t[:, sl * TAPW + c0:sl * TAPW + c1],
                                start=(first and sl == 0),
                                stop=(last and sl == K - 1))
                    done.add((g, mt))
                    if all((gg, mt) in done for gg in range(K)):
                        # out = num / den, as soon as this tile's taps finish
                        rden = ring.tile([128, NPOS], F32, tag="rden", bufs=2,
                                         name=f"rden{mt}")
                        nc.vector.reciprocal(rden[:], dn_ps[mt][:, :NPOS])
                        o_t = ring.tile([128, NPOS], F32, tag="o", bufs=2,
                                        name=f"o{mt}")
                        nc.vector.tensor_tensor(o_t[:], dn_ps[mt][:, NPOS:],
                                                rden[:], MULT)
                        nc.sync.dma_start(out_d[mt * 128:(mt + 1) * 128, :],
                                          o_t[:])


def _build_graph(repeat=1):
    nc = bacc.Bacc("TRN2", target_bir_lowering=False, debug=False,
                   num_devices=N_CORES)

    GDT = F32R if GEMM_F32R else F32
    dram = (
        nc.declare_dram_parameter("x_slab", [CIN, NPAD], GDT, isOutput=False),
        nc.declare_dram_parameter("w_qT", [CIN, COUT], GDT, isOutput=False),
        nc.declare_dram_parameter("w_kT", [CIN, COUT], GDT, isOutput=False),
        nc.declare_dram_parameter("w_vT", [CIN, COUT], GDT, isOutput=False),
        nc.declare_dram_parameter("beta_pk", [128, 2 * K], F32, isOutput=False),
        nc.declare_dram_parameter("ident", [128, 128], F32, isOutput=False),
        nc.declare_dram_parameter("out", [COUT, NPOS], F32, isOutput=True),
    )

    with tile.TileContext(nc) as tc:
        if repeat > 1:
            with tc.For_i(0, repeat, 1):
                _emit_body(nc, tc, dram)
        else:
            _emit_body(nc, tc, dram)

    nc.compile()
    return nc


def _prep_host(x, w_q, w_k, w_v, rel_h, rel_w):
    x = np.ascontiguousarray(x, np.float32)
    beta = np.zeros((COUT, K), np.float32)
    beta[:COUT // 2] = rel_h.reshape(COUT // 2, K)
    beta[COUT // 2:] = rel_w.reshape(COUT // 2, K)
    beta_pk = np.empty((128, 2 * K), np.float32)
    for mt in range(2):
        beta_pk[:, mt * K:(mt + 1) * K] = beta[mt * 128:(mt + 1) * 128]
    common = {
        "w_qT": np.ascontiguousarray(w_q.T, np.float32),
        "w_kT": np.ascontiguousarray(w_k.T, np.float32),
        "w_vT": np.ascontiguousarray(w_v.T, np.float32),
        "beta_pk": beta_pk,
        "ident": np.eye(128, dtype=np.float32),
    }
    in_maps = []
    for core in range(N_CORES):
        b, r0 = divmod(core, 4)
        r0 *= ROWS
        slab = np.zeros((CIN, SH, SW), np.float32)
        lo, hi = r0 - PAD, r0 + ROWS + PAD
        clo, chi = max(lo, 0), min(hi, H)
        slab[:, clo - lo:chi - lo, PAD:PAD + W] = x[b, :, clo:chi, :]
        in_maps.append({"x_slab": slab.reshape(CIN, NPAD), **common})
    return in_maps


def kernel(x, w_q, w_k, w_v, rel_h, rel_w):
    if "nc" not in _CACHED:
        _CACHED["nc"] = _build_graph()
    nc = _CACHED["nc"]
    in_maps = _prep_host(x, w_q, w_k, w_v, rel_h, rel_w)
    res = run_bass_kernel_spmd(nc, in_maps, core_ids=list(range(N_CORES)))
    _CACHED["exec_time_ns"] = res.exec_time_ns
    out = np.empty((B, COUT, H, W), np.float32)
    for core in range(N_CORES):
        b, r0 = divmod(core, 4)
        r0 *= ROWS
        out[b, :, r0:r0 + ROWS, :] = \
            res.results[core]["out"].reshape(COUT, ROWS, W)
    return out



# revision 28
# speedup vs baseline: 1.7814x; 1.7814x over previous
"""Trainium2 Bass kernel for local windowed per-channel attention (sparse_attention).

Reference computation (per batch b, channel c, position (h,w)):
    q = W_q x ; k = W_k x_pad ; v = W_v x_pad           (1x1 convs)
    s[i,j]  = q[h,w] * (k[h+i, w+j] + bias[c, i or j])  over a 7x7 window
    out     = sum_ij softmax_ij(s) * v[h+i, w+j]

Sharding: spatial, 8 ways — core = (batch, 12-row slab). Fully independent
per core (no collectives). Host pre-pads each slab with the 3-row/col halo.

Per-core dataflow (channels on partitions, 2 channel-tiles of 128):
  TensorE : q/k/v GEMMs in float32r (1 cyc/row); 49-tap reduction of
            den/num via identity-matmul accumulation into PSUM (bf16 rhs,
            f32 accumulate) — [e|m] interleaved per tap so den|num share
            one 3-bank region at 3 matmuls/tap.
  VectorE : score mult q*kb and weight mult e*v in fp16/bf16 at the DVE 2x
            perf mode, via shifted-window access patterns (no unfold
            materialization). The per-(c,tap) rel-pos bias is pre-folded
            into 7 biased fp16 k maps (channel-tile 0 biases by kh,
            tile 1 by kw) with 4x-mode tensor_scalar adds. Window taps
            are grouped so the bias slab is fixed per instruction, and
            split into even/odd kw sets reading 1-column-shifted copies
            (kb1/v1) so every innermost fp16 run stays 4B-aligned.
  ScalarE : exp, in place on the e slots (fp16 scores -> bf16), plus PSUM
            evictions.
  GpSimd  : a tuned slice of the weight-mult work (the score mults stay on
            VectorE — exp waits on them and Pool is ~3.4x slower).
  DMA     : shifted-copy production (kb1/v1) besides I/O.
  out = num / den  (reciprocal + mult), DMA out.
"""
import os
import numpy as np

from concourse import bass, bacc, mybir, tile
from concourse.bass_utils import run_bass_kernel_spmd

F32 = mybir.dt.float32
F16 = mybir.dt.float16
BF16 = mybir.dt.bfloat16

K, PAD = 7, 3
B, CIN, COUT, H, W = 2, 256, 256, 48, 48
ROWS = 12                 # output rows per core
SH, SW = ROWS + 2 * PAD, W + 2 * PAD   # 18, 54 padded slab
NPOS = ROWS * W           # 576 output positions per core
NPAD = SH * SW            # 972 padded positions
NQ = ROWS * SW            # 648 q-map positions (12 rows x 54 cols)
N_CORES = 8

SCORE_DT = F16            # kb, q, s dtype (f16 -> DVE 2x mode)
E_DT = BF16               # exp output / matmul rhs dtype (needs bf16 range)
V_DT = F16
F32R = mybir.dt.float32r  # TensorE fp32-replicated: 1 cyc/row when n>=256
GEMM_F32R = bool(int(os.environ.get("GEMM_F32R", "1")))
# m-mult groups offloaded to GpSimd (score mults stay on DVE: exp waits on
# them and Pool is ~3.4x slower per group). Keys: mt0 -> (i, par); mt1 -> j.
POOL_M0 = set((i, 1) for i in (0, 2, 4, 6))
# mt1 m-mults are emitted in two i-halves (finer Pool quanta): (g, half)
POOL_M1 = {(g, 0) for g in range(K)}
TAPW = 2 * NPOS           # per-tap [e | m] interleaved width
RING_BUFS = int(os.environ.get("RING_BUFS", "6"))
MT_ILV = bool(int(os.environ.get("MT_ILV", "0")))

JEVEN = [0, 2, 4, 6]
JODD = [1, 3, 5]

_CACHED = {}


def _fap(t, offset, dims):
    """Custom free-dim AP on a tile: dims = [[stride, size], ...]."""
    a = t[:]
    return bass.AP(a.tensor, a.offset + offset, [list(a.ap[0])] + dims)


def _emit_body(nc, tc, dram):
    x_d, wq_d, wk_d, wv_d, beta_d, id_d, out_d = dram
    MULT = mybir.AluOpType.mult
    with (
        tc.tile_pool(name="const", bufs=1) as const,
        tc.tile_pool(name="work", bufs=1) as work,
    ):
        # ---- load inputs (small/critical first) ----
        GDT = F32R if GEMM_F32R else F32
        beta_sb = const.tile([128, 2 * K], F32, name="beta_sb")
        nc.sync.dma_start(beta_sb[:], beta_d[:, :])
        idf = const.tile([128, 128], F32, name="idf")
        nc.sync.dma_start(idf[:], id_d[:, :])
        idb = const.tile([128, 128], E_DT, name="idb")
        nc.vector.tensor_copy(idb[:], idf[:])
        # warm the ACT exp table while DMAs stream in
        warm = const.tile([128, 2], F32, name="warm")
        nc.scalar.activation(warm[:], idf[:, :2],
                             mybir.ActivationFunctionType.Exp)
        x_sb = []
        for kt in range(2):
            t = const.tile([128, NPAD], GDT, name=f"x_sb{kt}")
            nc.sync.dma_start(t[:, :512], x_d[kt * 128:(kt + 1) * 128, :512])
            nc.sync.dma_start(t[:, 512:], x_d[kt * 128:(kt + 1) * 128, 512:])
            x_sb.append(t)
        w_sb = {}
        for nm, d in (("k", wk_d), ("q", wq_d), ("v", wv_d)):
            for kt in range(2):
                t = const.tile([128, COUT], GDT, name=f"w{nm}{kt}")
                nc.sync.dma_start(t[:], d[kt * 128:(kt + 1) * 128, :])
                w_sb[nm, kt] = t

        # ---- per channel-tile persistent tensors ----
        # kb0 = 7 pre-biased fp16 k maps (slab t: k + beta[c, t]); kb1 =
        # 1-col-shifted copy of the stack (odd-kw tap 4B alignment).
        k0 = [work.tile([128, NPAD], SCORE_DT, name=f"k0_{mt}")
              for mt in range(2)]
        kb0 = [work.tile([128, K * NPAD], SCORE_DT, name=f"kb0_{mt}")
               for mt in range(2)]
        kb1 = [work.tile([128, K * NPAD], SCORE_DT, name=f"kb1_{mt}")
               for mt in range(2)]
        v0 = [work.tile([128, NPAD], V_DT, name=f"v0_{mt}") for mt in range(2)]
        v1 = [work.tile([128, NPAD], V_DT, name=f"v1_{mt}") for mt in range(2)]
        q_sb = [work.tile([128, NPOS], SCORE_DT, name=f"q{mt}")
                for mt in range(2)]

        def gmm(out, lhsT, rhs, start, stop):
            nc.tensor.matmul(out, lhsT, rhs, start=start, stop=stop)

        # ---- GEMMs ----
        with tc.tile_pool(name="gpsum", bufs=3, space="PSUM") as gpsum:
            for mt in range(2):
                mm = slice(mt * 128, (mt + 1) * 128)
                # k map
                kp = gpsum.tile([128, NPAD], F32, tag="gp", name=f"kp{mt}")
                for kt in range(2):
                    for c0, c1 in ((0, 512), (512, NPAD)):
                        gmm(kp[:, c0:c1], w_sb["k", kt][:, mm],
                            x_sb[kt][:, c0:c1],
                            start=(kt == 0), stop=(kt == 1))
                nc.scalar.copy(k0[mt][:], kp[:])
                # biased stack: fp16 tensor_scalar adds run the 4x DVE mode;
                # per-slab shifted copies (DMA) unblock early tap groups
                for t in range(K):
                    nc.vector.tensor_scalar_add(
                        kb0[mt][:, t * NPAD:(t + 1) * NPAD], k0[mt][:],
                        beta_sb[:, mt * K + t:mt * K + t + 1])
                    nc.sync.dma_start(
                        kb1[mt][:, t * NPAD:t * NPAD + NPAD - 2],
                        kb0[mt][:, t * NPAD + 1:t * NPAD + NPAD - 1])
                # q map: only the 12 center rows (cols incl. pad)
                qp = gpsum.tile([128, NQ], F32, tag="gp", name=f"qp{mt}")
                for kt in range(2):
                    for c0, c1 in ((0, 512), (512, NQ)):
                        gmm(qp[:, c0:c1], w_sb["q", kt][:, mm],
                            x_sb[kt][:, PAD * SW + c0:PAD * SW + c1],
                            start=(kt == 0), stop=(kt == 1))
                # evict q to dense [128, 12*48] fp16 (drop col pad)
                nc.vector.tensor_copy(
                    q_sb[mt][:].rearrange("p (h w) -> p h w", h=ROWS),
                    _fap(qp, PAD, [[SW, ROWS], [1, W]]))
                # v map
                vp = gpsum.tile([128, NPAD], F32, tag="gp", name=f"vp{mt}")
                for kt in range(2):
                    for c0, c1 in ((0, 512), (512, NPAD)):
                        gmm(vp[:, c0:c1], w_sb["v", kt][:, mm],
                            x_sb[kt][:, c0:c1],
                            start=(kt == 0), stop=(kt == 1))
                nc.scalar.copy(v0[mt][:], vp[:])
                nc.sync.dma_start(v1[mt][:, :NPAD - 2],
                                  v0[mt][:, 1:NPAD - 1])

        # ---- attention ----
        # Per-tap layout em_t[:, j*TAPW + (0:NPOS | NPOS:TAPW)] = [e_j | m_j]
        # so den|num accumulate into ONE [128, TAPW] PSUM region (3 banks)
        # with 3 matmuls per tap (512 | 512 | 128 cols).
        with (
            tc.tile_pool(name="apsum", bufs=1, space="PSUM") as apsum,
            tc.tile_pool(name="ring", bufs=RING_BUFS) as ring,
        ):
            dn_ps = [apsum.tile([128, TAPW], F32, name=f"dn{mt}")
                     for mt in range(2)]
            if MT_ILV:
                order = [(g, mt) for g in range(K) for mt in range(2)]
            else:
                order = [(g, mt) for mt in range(2) for g in range(K)]
            done = set()
            SWPIPE = bool(int(os.environ.get("SWPIPE", "0")))
            if SWPIPE:
                # software-pipelined emission: s/exp of group n+1 emitted
                # ahead of m/matmuls of group n, so the scheduler prioritizes
                # feeding the ScalarE exp pacer.
                stages = []
                for idx, gm in enumerate(order):
                    stages.append(("se", gm))
                    if idx >= 1:
                        stages.append(("mm", order[idx - 1]))
                stages.append(("mm", order[-1]))
            else:
                stages = []
                for gm in order:
                    stages.append(("se", gm))
                    stages.append(("mm", gm))
            em_tiles = {}
            for stage, (g, mt) in stages:
                    # group g: kh=i for tile0, kw=j for tile1 (bias slab g).
                    # Scores are written straight into the e-slots of em_t
                    # (as fp16 via bitcast) and exp'd in place (bf16 out).
                    if stage == "mm":
                        em_t = em_tiles.pop((g, mt))
                    else:
                        em_t = ring.tile([128, K * TAPW], E_DT,
                                         tag=f"em{mt}", bufs=RING_BUFS // 2,
                                         name=f"em{mt}_{g}")
                        em_tiles[(g, mt)] = em_t
                    if stage == "mm":
                        pass
                    elif mt == 0:
                        # slots = j; even/odd kw split for fp16 alignment
                        for par, jl in ((0, JEVEN), (1, JODD)):
                            nj = len(jl)
                            kb_ap = _fap(kb1[0] if par else kb0[0],
                                         g * NPAD + g * SW + (jl[0] - par),
                                         [[2, nj], [SW, ROWS], [1, W]])
                            q_ap = _fap(q_sb[0], 0,
                                        [[0, nj], [W, ROWS], [1, W]])
                            s_ap = _fap(em_t, par * TAPW,
                                        [[2 * TAPW, nj], [W, ROWS], [1, W]]
                                        ).bitcast(SCORE_DT)
                            nc.vector.tensor_tensor(s_ap, kb_ap, q_ap, MULT)
                    else:
                        # slots = i; kw=g fixed -> one instr, parity by g
                        par = g % 2
                        kb_ap = _fap(kb1[1] if par else kb0[1],
                                     g * NPAD + (g - par),
                                     [[SW, K], [SW, ROWS], [1, W]])
                        q_ap = _fap(q_sb[1], 0, [[0, K], [W, ROWS], [1, W]])
                        s_ap = _fap(em_t, 0, [[TAPW, K], [W, ROWS], [1, W]]
                                    ).bitcast(SCORE_DT)
                        nc.vector.tensor_tensor(s_ap, kb_ap, q_ap, MULT)
                    if stage == "se":
                        # e = exp(s) in place on the e slots
                        nc.scalar.activation(
                            _fap(em_t, 0, [[TAPW, K], [1, NPOS]]),
                            _fap(em_t, 0, [[TAPW, K], [1, NPOS]]
                                 ).bitcast(SCORE_DT),
                            mybir.ActivationFunctionType.Exp)
                        continue
                    # m = e * v_shift -> m slots of em_t
                    if mt == 0:
                        for par, jl in ((0, JEVEN), (1, JODD)):
                            nj = len(jl)
                            e_ap = _fap(em_t, par * TAPW,
                                        [[2 * TAPW, nj], [W, ROWS], [1, W]])
                            v_ap = _fap(v1[0] if par else v0[0],
                                        g * SW + (jl[0] - par),
                                        [[2, nj], [SW, ROWS], [1, W]])
                            m_ap = _fap(em_t, par * TAPW + NPOS,
                                        [[2 * TAPW, nj], [W, ROWS], [1, W]])
                            eng = (nc.gpsimd if (g, par) in POOL_M0
                                   else nc.vector)
                            eng.tensor_tensor(m_ap, e_ap, v_ap, MULT)
                    else:
                        par = g % 2
                        for half, (i0, ni) in enumerate(((0, 4), (4, 3))):
                            e_ap = _fap(em_t, i0 * TAPW,
                                        [[TAPW, ni], [W, ROWS], [1, W]])
                            v_ap = _fap(v1[1] if par else v0[1],
                                        g - par + i0 * SW,
                                        [[SW, ni], [SW, ROWS], [1, W]])
                            m_ap = _fap(em_t, i0 * TAPW + NPOS,
                                        [[TAPW, ni], [W, ROWS], [1, W]])
                            eng = (nc.gpsimd if (g, half) in POOL_M1
                                   else nc.vector)
                            eng.tensor_tensor(m_ap, e_ap, v_ap, MULT)
                    # accumulate [den | num] += [e | m] via identity matmul
                    first = (g == 0)
                    last = (g == K - 1)
                    for sl in range(K):
                        for c0, c1 in ((0, 512), (512, 1024), (1024, TAPW)):
                            nc.tensor.matmul(
                                dn_ps[mt][:, c0:c1], idb[:],
                                em_t[:, sl * TAPW + c0:sl * TAPW + c1],
                                start=(first and sl == 0),
                                stop=(last and sl == K - 1))
                    done.add((g, mt))
                    if all((gg, mt) in done for gg in range(K)):
                        # out = num / den, as soon as this tile's taps finish
                        rden = ring.tile([128, NPOS], F32, tag="rden", bufs=2,
                                         name=f"rden{mt}")
                        nc.vector.reciprocal(rden[:], dn_ps[mt][:, :NPOS])
                        o_t = ring.tile([128, NPOS], F32, tag="o", bufs=2,
                                        name=f"o{mt}")
                        nc.vector.tensor_tensor(o_t[:], dn_ps[mt][:, NPOS:],
                                                rden[:], MULT)
                        nc.sync.dma_start(out_d[mt * 128:(mt + 1) * 128, :],
                                          o_t[:])


def _build_graph(repeat=1):
    nc = bacc.Bacc("TRN2", target_bir_lowering=False, debug=False,
                   num_devices=N_CORES)

    GDT = F32R if GEMM_F32R else F32
    dram = (
        nc.declare_dram_parameter("x_slab", [CIN, NPAD], GDT, isOutput=False),
        nc.declare_dram_parameter("w_qT", [CIN, COUT], GDT, isOutput=False),
        nc.declare_dram_parameter("w_kT", [CIN, COUT], GDT, isOutput=False),
        nc.declare_dram_parameter("w_vT", [CIN, COUT], GDT, isOutput=False),
        nc.declare_dram_parameter("beta_pk", [128, 2 * K], F32, isOutput=False),
        nc.declare_dram_parameter("ident", [128, 128], F32, isOutput=False),
        nc.declare_dram_parameter("out", [COUT, NPOS], F32, isOutput=True),
    )

    with tile.TileContext(nc) as tc:
        if repeat > 1:
            with tc.For_i(0, repeat, 1):
                _emit_body(nc, tc, dram)
        else:
            _emit_body(nc, tc, dram)

    nc.compile()
    return nc


def _prep_host(x, w_q, w_k, w_v, rel_h, rel_w):
    x = np.ascontiguousarray(x, np.float32)
    beta = np.zeros((COUT, K), np.float32)
    beta[:COUT // 2] = rel_h.reshape(COUT // 2, K)
    beta[COUT // 2:] = rel_w.reshape(COUT // 2, K)
    beta_pk = np.empty((128, 2 * K), np.float32)
    for mt in range(2):
        beta_pk[:, mt * K:(mt + 1) * K] = beta[mt * 128:(mt + 1) * 128]
    common = {
        "w_qT": np.ascontiguousarray(w_q.T, np.float32),
        "w_kT": np.ascontiguousarray(w_k.T, np.float32),
        "w_vT": np.ascontiguousarray(w_v.T, np.float32),
        "beta_pk": beta_pk,
        "ident": np.eye(128, dtype=np.float32),
    }
    in_maps = []
    for core in range(N_CORES):
        b, r0 = divmod(core, 4)
        r0 *= ROWS
        slab = np.zeros((CIN, SH, SW), np.float32)
        lo, hi = r0 - PAD, r0 + ROWS + PAD
        clo, chi = max(lo, 0), min(hi, H)
        slab[:, clo - lo:chi - lo, PAD:PAD + W] = x[b, :, clo:chi, :]
        in_maps.append({"x_slab": slab.reshape(CIN, NPAD), **common})
    return in_maps


def kernel(x, w_q, w_k, w_v, rel_h, rel_w):
    if "nc" not in _CACHED:
        _CACHED["nc"] = _build_graph()
    nc = _CACHED["nc"]
    in_maps = _prep_host(x, w_q, w_k, w_v, rel_h, rel_w)
    res = run_bass_kernel_spmd(nc, in_maps, core_ids=list(range(N_CORES)))
    _CACHED["exec_time_ns"] = res.exec_time_ns
    out = np.empty((B, COUT, H, W), np.float32)
    for core in range(N_CORES):
        b, r0 = divmod(core, 4)
        r0 *= ROWS
        out[b, :, r0:r0 + ROWS, :] = \
            res.results[core]["out"].reshape(COUT, ROWS, W)
    return out

